# revision 7
# baseline (speedup 1.0000x reference)
"""Multi-head attention block (QKV proj -> softmax attention -> out proj ->
residual + LayerNorm) on 8 Trainium2 NeuronCores, data-parallel over batch.

Shapes (hardcoded): B=8, S=1024, H=16, HD=64, D=1024.
Each core runs one batch element. All matmuls use float32r (~1.5e-4 rel err).

Dataflow per core (x_b [S,D]):
  xT   [D,S]   via PE transposes
  qT   = wq^T @ xT  (+bq), kT likewise        [D,S], head h = rows h*64..h*64+64
  v'   = x @ wv (+bv) with a ones column per head -> [S, 16*65]
  per head, per q-half:
    scoresT[k,q] = kT_h^T-slice matmuls (K=64)            -> PSUM [128,512]
    E = exp(0.125*scoresT + mask_k * -1e4)  (fused ACT)   -> SBUF f32r
    ctx' = v'_h^T @ E  accumulated over k-tiles           -> PSUM [65,512]
           rows 0..63 = unnormalized ctxT_h, row 64 = colsum
    ctxT_h = ctx'[0:64] * (1/colsum)  (partition_broadcast)
  proj = ctxT^T-slice @ wo (+bo), out = LayerNorm(x + proj) * gamma + beta
"""
import sys
import time

sys.path.insert(0, "/opt/trn_rl_repo")

import numpy as np

import concourse.bass as bass
import concourse.bacc as bacc
import concourse.tile as tile
from concourse import mybir
from concourse.bass_utils import run_bass_kernel_spmd
from concourse.masks import make_identity

F32 = mybir.dt.float32
F32R = mybir.dt.float32r
AF = mybir.ActivationFunctionType

B, S, H, HD = 8, 1024, 16, 64
D = H * HD
NINF = -10000.0
EPS = 1e-6
ST = S // 128   # 8 s-tiles
DT = D // 128   # 8 d-tiles
NH = S // 512   # 2 free-dim halves


def build_bass():
    nc = bacc.Bacc("TRN2", target_bir_lowering=False, debug=False)

    xb = nc.dram_tensor("xb", [S, D], F32, kind="ExternalInput").ap()
    maskneg = nc.dram_tensor("maskneg", [S], F32, kind="ExternalInput").ap()
    wq = nc.dram_tensor("wq", [D, D], F32R, kind="ExternalInput").ap()
    wk = nc.dram_tensor("wk", [D, D], F32R, kind="ExternalInput").ap()
    wv = nc.dram_tensor("wv", [D, D], F32R, kind="ExternalInput").ap()
    wo = nc.dram_tensor("wo", [D, D], F32R, kind="ExternalInput").ap()
    bq = nc.dram_tensor("bq", [D], F32, kind="ExternalInput").ap()
    bk = nc.dram_tensor("bk", [D], F32, kind="ExternalInput").ap()
    bv = nc.dram_tensor("bv", [D], F32, kind="ExternalInput").ap()
    bo = nc.dram_tensor("bo", [D], F32, kind="ExternalInput").ap()
    gamma = nc.dram_tensor("gamma", [D], F32, kind="ExternalInput").ap()
    beta = nc.dram_tensor("beta", [D], F32, kind="ExternalInput").ap()
    out = nc.dram_tensor("out", [S, D], F32, kind="ExternalOutput").ap()

    with tile.TileContext(nc) as tc:
        with (
            tc.tile_pool(name="const", bufs=1) as const,
            tc.tile_pool(name="persist", bufs=1) as persist,
        ):
            qT = persist.tile([128, DT, S], F32R)
            kT = persist.tile([128, DT, S], F32R)
            vp = persist.tile([128, ST, H * (HD + 1)], F32R)

            ident = const.tile([128, 128], F32)
            make_identity(nc, ident)
            bq_sb = const.tile([128, DT], F32)
            nc.sync.dma_start(out=bq_sb, in_=bq.rearrange("(t p) -> p t", p=128))
            bk_sb = const.tile([128, DT], F32)
            nc.sync.dma_start(out=bk_sb, in_=bk.rearrange("(t p) -> p t", p=128))
            mneg_sb = const.tile([128, ST], F32)
            nc.sync.dma_start(out=mneg_sb, in_=maskneg.rearrange("(t p) -> p t", p=128))
            def part_bcast(v):
                return bass.AP(tensor=v.tensor, offset=v.offset,
                               ap=[[0, 128]] + list(v.ap))

            bo_b = const.tile([128, D], F32)
            nc.gpsimd.dma_start(out=bo_b, in_=part_bcast(bo))
            gamma_b = const.tile([128, D], F32)
            nc.gpsimd.dma_start(out=gamma_b, in_=part_bcast(gamma))
            beta_b = const.tile([128, D], F32)
            nc.gpsimd.dma_start(out=beta_b, in_=part_bcast(beta))
            eps_sb = const.tile([128, 1], F32)
            nc.vector.memset(eps_sb, EPS)
            ones16 = const.tile([128, H], F32)
            nc.vector.memset(ones16, 1.0)

            # ---------------- phase 1: xT, qT, kT, v' ----------------
            with (
                tc.tile_pool(name="xTp", bufs=1) as xTp,
                tc.tile_pool(name="ph1", bufs=3) as ph1,
                tc.tile_pool(name="wst", bufs=2) as wst,
                tc.tile_pool(name="p1c", bufs=1) as p1c,
                tc.tile_pool(name="ps1", bufs=3, space="PSUM") as ps1,
                tc.tile_pool(name="ps1t", bufs=3, space="PSUM") as ps1t,
            ):
                bv_b = p1c.tile([128, D], F32)
                nc.gpsimd.dma_start(out=bv_b, in_=part_bcast(bv))
                xT = xTp.tile([128, DT, S], F32R)
                for st in range(ST):
                    x_t = ph1.tile([128, D], F32, tag="x_t")
                    nc.sync.dma_start(out=x_t, in_=xb[st * 128:(st + 1) * 128, :])
                    for dt in range(DT):
                        tp = ps1t.tile([128, 128], F32, tag="tp")
                        nc.tensor.transpose(tp, x_t[:, dt * 128:(dt + 1) * 128], ident)
                        nc.vector.tensor_copy(
                            xT[:, dt, st * 128:(st + 1) * 128], tp)

                # ones columns of v'
                vp_h = vp.rearrange("p t (h j) -> p t h j", j=HD + 1)
                for st in range(ST):
                    nc.vector.tensor_copy(vp_h[:, st, :, HD], ones16)

                # qT / kT: out[d_out, s] = w^T @ xT, bias per-partition
                for wdram, bsb, dstT in ((wq, bq_sb, qT), (wk, bk_sb, kT)):
                    for cs in range(2):
                        wcs = wst.tile([128, DT, 512], F32R, tag="wcs")
                        nc.sync.dma_start(
                            out=wcs,
                            in_=wdram[:, cs * 512:(cs + 1) * 512].rearrange(
                                "(t p) n -> p t n", p=128))
                        for mt in range(4):
                            mg = cs * 4 + mt
                            for sh in range(NH):
                                qps = ps1.tile([128, 512], F32, tag="qps")
                                for kt in range(DT):
                                    nc.tensor.matmul(
                                        qps,
                                        wcs[:, kt, mt * 128:(mt + 1) * 128],
                                        xT[:, kt, sh * 512:(sh + 1) * 512],
                                        start=(kt == 0), stop=(kt == DT - 1))
                                nc.vector.tensor_scalar_add(
                                    dstT[:, mg, sh * 512:(sh + 1) * 512],
                                    qps, bsb[:, mg:mg + 1])

                # v natural with bias, scattered into v' head-columns
                for et in range(2):
                    wcs = wst.tile([128, DT, 512], F32R, tag="wcs")
                    nc.sync.dma_start(
                        out=wcs,
                        in_=wv[:, et * 512:(et + 1) * 512].rearrange(
                            "(t p) n -> p t n", p=128))
                    for st in range(ST):
                        vps = ps1.tile([128, 512], F32, tag="qps")
                        for kt in range(DT):
                            nc.tensor.matmul(
                                vps,
                                xT[:, kt, st * 128:(st + 1) * 128],
                                wcs[:, kt, :],
                                start=(kt == 0), stop=(kt == DT - 1))
                        for hh in range(8):
                            h = et * 8 + hh
                            nc.vector.scalar_tensor_tensor(
                                out=vp[:, st, h * (HD + 1):h * (HD + 1) + HD],
                                in0=vps[:, hh * HD:(hh + 1) * HD],
                                scalar=1.0,
                                in1=bv_b[:, h * HD:(h + 1) * HD],
                                op0=mybir.AluOpType.mult,
                                op1=mybir.AluOpType.add)

            # ---------------- phase 2: attention + proj + LN ----------------
            with (
                tc.tile_pool(name="Ep", bufs=3) as Ep,
                tc.tile_pool(name="ctxTp", bufs=2) as ctxTp,
                tc.tile_pool(name="wop", bufs=1) as wop,
                tc.tile_pool(name="xep", bufs=1) as xep,
                tc.tile_pool(name="epi", bufs=2) as epi,
                tc.tile_pool(name="nrm", bufs=2) as nrm,
                tc.tile_pool(name="scps", bufs=3, space="PSUM") as scps,
                tc.tile_pool(name="ctxps", bufs=2, space="PSUM") as ctxps,
                tc.tile_pool(name="pjps", bufs=1, space="PSUM") as pjps,
            ):
                for qh in range(NH):
                    ctxT = ctxTp.tile([128, DT, 512], F32R, tag="ctxT")
                    for h in range(H):
                        base = (h % 2) * 64
                        dt = h // 2
                        ctx_ps = ctxps.tile([HD + 1, 512], F32, tag="ctx")
                        for kt in range(ST):
                            scp = scps.tile([128, 512], F32, tag="scp")
                            nc.tensor.matmul(
                                scp,
                                kT[base:base + 64, dt, kt * 128:(kt + 1) * 128],
                                qT[base:base + 64, dt, qh * 512:(qh + 1) * 512],
                                start=True, stop=True)
                            e_t = Ep.tile([128, 512], F32R, tag="E")
                            nc.scalar.activation(
                                e_t, scp, AF.Exp,
                                bias=mneg_sb[:, kt:kt + 1], scale=0.125)
                            nc.tensor.matmul(
                                ctx_ps,
                                vp[:, kt, h * (HD + 1):(h + 1) * (HD + 1)],
                                e_t,
                                start=(kt == 0), stop=(kt == ST - 1))
                        rrow = nrm.tile([HD + 1, 512], F32, tag="rr")
                        nc.vector.reciprocal(rrow[0:1, :], ctx_ps[64:65, :])
                        rbc = nrm.tile([64, 512], F32, tag="rbc")
                        nc.gpsimd.partition_broadcast(rbc, rrow[0:1, :])
                        nc.vector.tensor_mul(
                            ctxT[base:base + 64, dt, :], ctx_ps[0:64, :], rbc)

                    # output projection + epilogue for this q-half
                    wo_cs0 = wop.tile([128, DT, 512], F32R, tag="wo0")
                    nc.sync.dma_start(
                        out=wo_cs0,
                        in_=wo[:, 0:512].rearrange("(t p) n -> p t n", p=128))
                    wo_cs1 = wop.tile([128, DT, 512], F32R, tag="wo1")
                    nc.sync.dma_start(
                        out=wo_cs1,
                        in_=wo[:, 512:1024].rearrange("(t p) n -> p t n", p=128))
                    for qt in range(4):
                        stg = qh * 4 + qt
                        pp = []
                        for et, wo_cs in enumerate((wo_cs0, wo_cs1)):
                            pps = pjps.tile([128, 512], F32, tag=f"pj{et}")
                            for dt in range(DT):
                                nc.tensor.matmul(
                                    pps,
                                    ctxT[:, dt, qt * 128:(qt + 1) * 128],
                                    wo_cs[:, dt, :],
                                    start=(dt == 0), stop=(dt == DT - 1))
                            pp.append(pps)
                        x_t = xep.tile([128, D], F32, tag="xe")
                        nc.sync.dma_start(
                            out=x_t, in_=xb[stg * 128:(stg + 1) * 128, :])
                        t = epi.tile([128, D], F32, tag="t")
                        for et in range(2):
                            nc.vector.scalar_tensor_tensor(
                                out=t[:, et * 512:(et + 1) * 512],
                                in0=pp[et], scalar=1.0,
                                in1=bo_b[:, et * 512:(et + 1) * 512],
                                op0=mybir.AluOpType.mult,
                                op1=mybir.AluOpType.add)
                        nc.vector.tensor_add(t, t, x_t)
                        stats = epi.tile(
                            [128, 2, nc.vector.BN_STATS_DIM], F32, tag="stats")
                        tg = t.rearrange("p (g d) -> p g d", g=2)
                        for g in range(2):
                            nc.vector.bn_stats(stats[:, g, :], tg[:, g, :])
                        mv = epi.tile([128, nc.vector.BN_AGGR_DIM], F32, tag="mv")
                        nc.vector.bn_aggr(mv, stats)
                        rstd = epi.tile([128, 1], F32, tag="rstd")
                        nc.scalar.activation(
                            rstd, mv[:, 1:2], AF.Sqrt, bias=eps_sb, scale=1.0)
                        nc.vector.reciprocal(rstd, rstd)
                        nc.vector.tensor_scalar(
                            t, t, mv[:, 0:1], rstd,
                            op0=mybir.AluOpType.subtract,
                            op1=mybir.AluOpType.mult)
                        ot = epi.tile([128, D], F32, tag="ot")
                        nc.vector.tensor_mul(ot, t, gamma_b)
                        nc.vector.tensor_add(ot, ot, beta_b)
                        nc.sync.dma_start(
                            out=out[stg * 128:(stg + 1) * 128, :], in_=ot)

    nc.compile()
    return nc


_NC_CACHE = []


def _get_nc():
    if not _NC_CACHE:
        _NC_CACHE.append(build_bass())
    return _NC_CACHE[0]


def make_in_maps(x, mask, wq, bq, wk, bk, wv, bv, wo, bo, gamma, beta):
    x = np.asarray(x, dtype=np.float32)
    mask = np.asarray(mask)
    maskneg = (mask.astype(np.float32) * NINF).astype(np.float32)
    common = {
        "wq": np.asarray(wq, np.float32), "wk": np.asarray(wk, np.float32),
        "wv": np.asarray(wv, np.float32), "wo": np.asarray(wo, np.float32),
        "bq": np.asarray(bq, np.float32), "bk": np.asarray(bk, np.float32),
        "bv": np.asarray(bv, np.float32), "bo": np.asarray(bo, np.float32),
        "gamma": np.asarray(gamma, np.float32),
        "beta": np.asarray(beta, np.float32),
    }
    return [dict(common, xb=np.ascontiguousarray(x[c]),
                 maskneg=np.ascontiguousarray(maskneg[c])) for c in range(B)]


def kernel(x, mask, wq, bq, wk, bk, wv, bv, wo, bo, gamma, beta):
    nc = _get_nc()
    in_maps = make_in_maps(x, mask, wq, bq, wk, bk, wv, bv, wo, bo, gamma, beta)
    last_err = None
    for _ in range(3):
        try:
            res = run_bass_kernel_spmd(nc, in_maps, core_ids=list(range(B)))
            return np.stack([res.results[c]["out"] for c in range(B)], axis=0)
        except Exception as e:  # transient NRT device errors: retry
            last_err = e
            time.sleep(5)
    raise last_err


# revision 9
# speedup vs baseline: 80.1890x; 80.1890x over previous
"""Multi-head attention block (QKV proj -> softmax attention -> out proj ->
residual + LayerNorm) on 8 Trainium2 NeuronCores, data-parallel over batch.

Shapes (hardcoded): B=8, S=1024, H=16, HD=64, D=1024.
Each core runs one batch element. All matmuls use float32r (~1.5e-4 rel err).

Dataflow per core (x_b [S,D]):
  xT   [D,S]   via PE transposes
  qT   = wq^T @ xT  (+bq), kT likewise        [D,S], head h = rows h*64..h*64+64
  v'   = x @ wv (+bv) with a ones column per head -> [S, 16*65]
  per head, per q-half:
    scoresT[k,q] = kT_h-slice matmuls (K=64)              -> PSUM [128,512]
    E = exp(0.125*scoresT + mask_k * -1e4)  (fused ACT)   -> SBUF f32r
    ctx' = v'_h^T @ E  accumulated over k-tiles           -> PSUM [65,512]
           rows 0..63 = unnormalized ctxT_h, row 64 = colsum
    ctxT_h = ctx'[0:64] * (1/colsum)  (partition_broadcast)
  proj = ctxT-slice @ wo (+bo), out = LayerNorm(x + proj) * gamma + beta
"""
import sys
import time

sys.path.insert(0, "/opt/trn_rl_repo")

import numpy as np

import concourse.bass as bass
import concourse.bacc as bacc
import concourse.tile as tile
from concourse import mybir
from concourse.bass_utils import run_bass_kernel_spmd
from concourse.masks import make_identity

F32 = mybir.dt.float32
F32R = mybir.dt.float32r
AF = mybir.ActivationFunctionType

B, S, H, HD = 8, 1024, 16, 64
D = H * HD
NINF = -10000.0
EPS = 1e-6
ST = S // 128   # 8 s-tiles
DT = D // 128   # 8 d-tiles
NH = S // 512   # 2 free-dim halves


def _emit_body(nc, tc, io, cst):
    """Emit one full forward pass."""
    xb, maskneg, wq, wk, wv, wo, bq, bk, bv, bo, gamma, beta, out = io
    (ident, bq_sb, bk_sb, mneg_sb, bo_b, gamma_b, beta_b, eps_sb, ones16,
     part_bcast) = cst

    with (
        tc.tile_pool(name="persist", bufs=1) as persist,
    ):
        qT = persist.tile([128, DT, S], F32R, name="qT")
        kT = persist.tile([128, DT, S], F32R, name="kT")
        vp = persist.tile([128, ST, H * (HD + 1)], F32R, name="vp")

        # ---------------- phase 1: xT, qT, kT, v' ----------------
        with (
            tc.tile_pool(name="xTp", bufs=1) as xTp,
            tc.tile_pool(name="ph1", bufs=3) as ph1,
            tc.tile_pool(name="wst", bufs=2) as wst,
            tc.tile_pool(name="p1c", bufs=1) as p1c,
            tc.tile_pool(name="ps1", bufs=3, space="PSUM") as ps1,
            tc.tile_pool(name="ps1t", bufs=3, space="PSUM") as ps1t,
        ):
            bv_b = p1c.tile([128, D], F32, name="bv_b")
            nc.gpsimd.dma_start(out=bv_b, in_=part_bcast(bv))
            xT = xTp.tile([128, DT, S], F32R, name="xT")
            for st in range(ST):
                x_t = ph1.tile([128, D], F32, tag="x_t", name="x_t")
                nc.sync.dma_start(out=x_t, in_=xb[st * 128:(st + 1) * 128, :])
                for dt in range(DT):
                    tp = ps1t.tile([128, 128], F32, tag="tp", name="tp")
                    nc.tensor.transpose(tp, x_t[:, dt * 128:(dt + 1) * 128],
                                        ident)
                    nc.vector.tensor_copy(xT[:, dt, st * 128:(st + 1) * 128],
                                          tp)

            # ones columns of v'
            vp_h = vp.rearrange("p t (h j) -> p t h j", j=HD + 1)
            for st in range(ST):
                nc.vector.tensor_copy(vp_h[:, st, :, HD], ones16)

            # qT / kT: out[d_out, s] = w^T @ xT, bias per-partition
            for wdram, bsb, dstT in ((wq, bq_sb, qT), (wk, bk_sb, kT)):
                for cs in range(2):
                    wcs = wst.tile([128, DT, 512], F32R, tag="wcs", name="wcs")
                    nc.sync.dma_start(
                        out=wcs,
                        in_=wdram[:, cs * 512:(cs + 1) * 512].rearrange(
                            "(t p) n -> p t n", p=128))
                    for mt in range(4):
                        mg = cs * 4 + mt
                        for sh in range(NH):
                            qps = ps1.tile([128, 512], F32, tag="qps",
                                           name="qps")
                            for kt in range(DT):
                                nc.tensor.matmul(
                                    qps,
                                    wcs[:, kt, mt * 128:(mt + 1) * 128],
                                    xT[:, kt, sh * 512:(sh + 1) * 512],
                                    start=(kt == 0), stop=(kt == DT - 1))
                            nc.vector.tensor_scalar_add(
                                dstT[:, mg, sh * 512:(sh + 1) * 512],
                                qps, bsb[:, mg:mg + 1])

            # v natural with bias, scattered into v' head-columns
            for et in range(2):
                wcs = wst.tile([128, DT, 512], F32R, tag="wcs", name="wcs")
                nc.sync.dma_start(
                    out=wcs,
                    in_=wv[:, et * 512:(et + 1) * 512].rearrange(
                        "(t p) n -> p t n", p=128))
                for st in range(ST):
                    vps = ps1.tile([128, 512], F32, tag="qps", name="vps")
                    for kt in range(DT):
                        nc.tensor.matmul(
                            vps,
                            xT[:, kt, st * 128:(st + 1) * 128],
                            wcs[:, kt, :],
                            start=(kt == 0), stop=(kt == DT - 1))
                    for hh in range(8):
                        h = et * 8 + hh
                        nc.vector.scalar_tensor_tensor(
                            out=vp[:, st, h * (HD + 1):h * (HD + 1) + HD],
                            in0=vps[:, hh * HD:(hh + 1) * HD],
                            scalar=1.0,
                            in1=bv_b[:, h * HD:(h + 1) * HD],
                            op0=mybir.AluOpType.mult,
                            op1=mybir.AluOpType.add)

        # ---------------- phase 2: attention + proj + LN ----------------
        with (
            tc.tile_pool(name="Ep", bufs=3) as Ep,
            tc.tile_pool(name="ctxTp", bufs=2) as ctxTp,
            tc.tile_pool(name="wop", bufs=1) as wop,
            tc.tile_pool(name="xep", bufs=1) as xep,
            tc.tile_pool(name="epi", bufs=2) as epi,
            tc.tile_pool(name="nrm", bufs=2) as nrm,
            tc.tile_pool(name="scps", bufs=3, space="PSUM") as scps,
            tc.tile_pool(name="ctxps", bufs=2, space="PSUM") as ctxps,
            tc.tile_pool(name="pjps", bufs=1, space="PSUM") as pjps,
        ):
            for qh in range(NH):
                ctxT = ctxTp.tile([128, DT, 512], F32R, tag="ctxT",
                                  name="ctxT")
                for h in range(H):
                    base = (h % 2) * 64
                    dt = h // 2
                    ctx_ps = ctxps.tile([HD + 1, 512], F32, tag="ctx",
                                        name="ctx_ps")
                    for kt in range(ST):
                        scp = scps.tile([128, 512], F32, tag="scp", name="scp")
                        nc.tensor.matmul(
                            scp,
                            kT[base:base + 64, dt, kt * 128:(kt + 1) * 128],
                            qT[base:base + 64, dt, qh * 512:(qh + 1) * 512],
                            start=True, stop=True)
                        e_t = Ep.tile([128, 512], F32R, tag="E", name="e_t")
                        nc.scalar.activation(
                            e_t, scp, AF.Exp,
                            bias=mneg_sb[:, kt:kt + 1], scale=0.125)
                        nc.tensor.matmul(
                            ctx_ps,
                            vp[:, kt, h * (HD + 1):(h + 1) * (HD + 1)],
                            e_t,
                            start=(kt == 0), stop=(kt == ST - 1))
                    rrow = nrm.tile([HD + 1, 512], F32, tag="rr", name="rrow")
                    nc.vector.reciprocal(rrow[0:1, :], ctx_ps[64:65, :])
                    rbc = nrm.tile([64, 512], F32, tag="rbc", name="rbc")
                    nc.gpsimd.partition_broadcast(rbc, rrow[0:1, :])
                    nc.vector.tensor_mul(
                        ctxT[base:base + 64, dt, :], ctx_ps[0:64, :], rbc)

                # output projection + epilogue for this q-half
                wo_cs0 = wop.tile([128, DT, 512], F32R, tag="wo0", name="wo0")
                nc.sync.dma_start(
                    out=wo_cs0,
                    in_=wo[:, 0:512].rearrange("(t p) n -> p t n", p=128))
                wo_cs1 = wop.tile([128, DT, 512], F32R, tag="wo1", name="wo1")
                nc.sync.dma_start(
                    out=wo_cs1,
                    in_=wo[:, 512:1024].rearrange("(t p) n -> p t n", p=128))
                for qt in range(4):
                    stg = qh * 4 + qt
                    pp = []
                    for et, wo_cs in enumerate((wo_cs0, wo_cs1)):
                        pps = pjps.tile([128, 512], F32, tag=f"pj{et}",
                                        name="pps")
                        for dt in range(DT):
                            nc.tensor.matmul(
                                pps,
                                ctxT[:, dt, qt * 128:(qt + 1) * 128],
                                wo_cs[:, dt, :],
                                start=(dt == 0), stop=(dt == DT - 1))
                        pp.append(pps)
                    x_t = xep.tile([128, D], F32, tag="xe", name="x_e")
                    nc.sync.dma_start(
                        out=x_t, in_=xb[stg * 128:(stg + 1) * 128, :])
                    t = epi.tile([128, D], F32, tag="t", name="t")
                    for et in range(2):
                        nc.vector.scalar_tensor_tensor(
                            out=t[:, et * 512:(et + 1) * 512],
                            in0=pp[et], scalar=1.0,
                            in1=bo_b[:, et * 512:(et + 1) * 512],
                            op0=mybir.AluOpType.mult,
                            op1=mybir.AluOpType.add)
                    nc.vector.tensor_add(t, t, x_t)
                    stats = epi.tile([128, 2, nc.vector.BN_STATS_DIM], F32,
                                     tag="stats", name="stats")
                    tg = t.rearrange("p (g d) -> p g d", g=2)
                    for g in range(2):
                        nc.vector.bn_stats(stats[:, g, :], tg[:, g, :])
                    mv = epi.tile([128, nc.vector.BN_AGGR_DIM], F32, tag="mv",
                                  name="mv")
                    nc.vector.bn_aggr(mv, stats)
                    rstd = epi.tile([128, 1], F32, tag="rstd", name="rstd")
                    nc.scalar.activation(
                        rstd, mv[:, 1:2], AF.Sqrt, bias=eps_sb, scale=1.0)
                    nc.vector.reciprocal(rstd, rstd)
                    nc.vector.tensor_scalar(
                        t, t, mv[:, 0:1], rstd,
                        op0=mybir.AluOpType.subtract,
                        op1=mybir.AluOpType.mult)
                    ot = epi.tile([128, D], F32, tag="ot", name="ot")
                    nc.vector.tensor_mul(ot, t, gamma_b)
                    nc.vector.tensor_add(ot, ot, beta_b)
                    nc.sync.dma_start(
                        out=out[stg * 128:(stg + 1) * 128, :], in_=ot)


def build_bass(reps=1):
    nc = bacc.Bacc("TRN2", target_bir_lowering=False, debug=False)

    xb = nc.dram_tensor("xb", [S, D], F32, kind="ExternalInput").ap()
    maskneg = nc.dram_tensor("maskneg", [S], F32, kind="ExternalInput").ap()
    wq = nc.dram_tensor("wq", [D, D], F32R, kind="ExternalInput").ap()
    wk = nc.dram_tensor("wk", [D, D], F32R, kind="ExternalInput").ap()
    wv = nc.dram_tensor("wv", [D, D], F32R, kind="ExternalInput").ap()
    wo = nc.dram_tensor("wo", [D, D], F32R, kind="ExternalInput").ap()
    bq = nc.dram_tensor("bq", [D], F32, kind="ExternalInput").ap()
    bk = nc.dram_tensor("bk", [D], F32, kind="ExternalInput").ap()
    bv = nc.dram_tensor("bv", [D], F32, kind="ExternalInput").ap()
    bo = nc.dram_tensor("bo", [D], F32, kind="ExternalInput").ap()
    gamma = nc.dram_tensor("gamma", [D], F32, kind="ExternalInput").ap()
    beta = nc.dram_tensor("beta", [D], F32, kind="ExternalInput").ap()
    out = nc.dram_tensor("out", [S, D], F32, kind="ExternalOutput").ap()
    io = (xb, maskneg, wq, wk, wv, wo, bq, bk, bv, bo, gamma, beta, out)

    with tile.TileContext(nc) as tc:
        with tc.tile_pool(name="const", bufs=1) as const:
            ident = const.tile([128, 128], F32, name="ident")
            make_identity(nc, ident)
            bq_sb = const.tile([128, DT], F32, name="bq_sb")
            nc.sync.dma_start(out=bq_sb,
                              in_=bq.rearrange("(t p) -> p t", p=128))
            bk_sb = const.tile([128, DT], F32, name="bk_sb")
            nc.sync.dma_start(out=bk_sb,
                              in_=bk.rearrange("(t p) -> p t", p=128))
            mneg_sb = const.tile([128, ST], F32, name="mneg_sb")
            nc.sync.dma_start(out=mneg_sb,
                              in_=maskneg.rearrange("(t p) -> p t", p=128))

            def part_bcast(v):
                return bass.AP(tensor=v.tensor, offset=v.offset,
                               ap=[[0, 128]] + list(v.ap))

            bo_b = const.tile([128, D], F32, name="bo_b")
            nc.gpsimd.dma_start(out=bo_b, in_=part_bcast(bo))
            gamma_b = const.tile([128, D], F32, name="gamma_b")
            nc.gpsimd.dma_start(out=gamma_b, in_=part_bcast(gamma))
            beta_b = const.tile([128, D], F32, name="beta_b")
            nc.gpsimd.dma_start(out=beta_b, in_=part_bcast(beta))
            eps_sb = const.tile([128, 1], F32, name="eps_sb")
            nc.vector.memset(eps_sb, EPS)
            ones16 = const.tile([128, H], F32, name="ones16")
            nc.vector.memset(ones16, 1.0)

            cst = (ident, bq_sb, bk_sb, mneg_sb, bo_b, gamma_b, beta_b,
                   eps_sb, ones16, part_bcast)
            for _ in range(reps):
                _emit_body(nc, tc, io, cst)

    nc.compile()
    return nc


_NC_CACHE = {}


def _get_nc(reps=1):
    if reps not in _NC_CACHE:
        _NC_CACHE[reps] = build_bass(reps)
    return _NC_CACHE[reps]


def make_in_maps(x, mask, wq, bq, wk, bk, wv, bv, wo, bo, gamma, beta):
    x = np.asarray(x, dtype=np.float32)
    mask = np.asarray(mask)
    maskneg = (mask.astype(np.float32) * NINF).astype(np.float32)
    common = {
        "wq": np.asarray(wq, np.float32), "wk": np.asarray(wk, np.float32),
        "wv": np.asarray(wv, np.float32), "wo": np.asarray(wo, np.float32),
        "bq": np.asarray(bq, np.float32), "bk": np.asarray(bk, np.float32),
        "bv": np.asarray(bv, np.float32), "bo": np.asarray(bo, np.float32),
        "gamma": np.asarray(gamma, np.float32),
        "beta": np.asarray(beta, np.float32),
    }
    return [dict(common, xb=np.ascontiguousarray(x[c]),
                 maskneg=np.ascontiguousarray(maskneg[c])) for c in range(B)]


def kernel(x, mask, wq, bq, wk, bk, wv, bv, wo, bo, gamma, beta):
    nc = _get_nc()
    in_maps = make_in_maps(x, mask, wq, bq, wk, bk, wv, bv, wo, bo, gamma, beta)
    last_err = None
    for _ in range(3):
        try:
            res = run_bass_kernel_spmd(nc, in_maps, core_ids=list(range(B)))
            return np.stack([res.results[c]["out"] for c in range(B)], axis=0)
        except Exception as e:  # transient NRT device errors: retry
            last_err = e
            time.sleep(5)
    raise last_err
